# revision 15
# baseline (speedup 1.0000x reference)
"""Multi-head causal attention with RoPE on 8 TRN2 NeuronCores.

Sharding: 2 heads per core (head-parallel QKV + attention), then two
head-split AllToAlls regroup the context to t-sharded cores for the
output projection. All matmuls run in float32r (reduced-precision fp32,
1 cycle/row at N>=512 vs 4 for fp32; measured max rel err ~1.5e-4 on a
K=2048 contraction).

Layouts (per core, heads hg = 2i, 2i+1):
  qd/kd  SBUF [128=d, 2*4096]   head hl at cols [hl*4096 + t], t = b*2048+pos
                                 partitions = [even rope dims; odd rope dims]
                                 (host permutes Wq/Wk columns so this holds)
  vs     SBUF [128=t%128, 32*256] t-block tbg at cols [tbg*256 + (hl*128+dv)]
  scores S^T  PSUM [t=128, r=512] -> exp -> P^T f32r in SBUF
  ctx^T  PSUM [dv=128, r=512]     accumulated over t-blocks; denom via
                                  ones-matmul [128,512] (all rows equal)
  out^T  [oc=2048, my 512 t]      host concatenates + transposes
"""
import sys

if '/opt/trn_rl_repo' not in sys.path:
    sys.path.insert(0, '/opt/trn_rl_repo')

import numpy as np
import ml_dtypes
import concourse.bass as bass  # noqa: F401  (registers bass types)
import concourse.bacc as bacc
import concourse.mybir as mybir
import concourse.tile as tile
from concourse import bass_utils

B, T, D, H, DH = 2, 2048, 2048, 16, 128
NCORES = 8
HPC = H // NCORES          # heads per core = 2
DC = HPC * DH              # output cols per core for q/k/v = 256
BT = B * T                 # 4096
TS = 512                   # t-super / r-super tile
NTS = BT // TS             # 8
KC = D // 128              # 16 contraction chunks
NRS = T // TS              # 4 r-supers per (b, h) pair
SCALE = 1.0 / float(np.sqrt(DH))
ROPE_THETA = 10000.0

_cache = {}


def build(dbg=False):
    key = ('nc', dbg)
    if key in _cache:
        return _cache[key]
    dtr = mybir.dt.float32r
    dtf = mybir.dt.float32
    nc = bacc.Bacc("TRN2", target_bir_lowering=False, debug=False,
                   num_devices=NCORES)
    dbg_t = {}
    if dbg:
        for nm, shp in (("dqd", [128, HPC * BT]), ("dkd", [128, HPC * BT]),
                        ("dvs", [128, (BT // 128) * DC]),
                        ("dsend1", [NCORES * 128, TS]),
                        ("dsend2", [NCORES * 128, TS]),
                        ("drecv1", [NCORES * 128, TS]),
                        ("drecv2", [NCORES * 128, TS])):
            dbg_t[nm] = nc.dram_tensor(nm, shp, dtf, kind="ExternalOutput").ap()

    xT = nc.dram_tensor("xT", [D, BT], dtr, kind="ExternalInput").ap()
    wq = nc.dram_tensor("wq", [D, DC], dtr, kind="ExternalInput").ap()
    wk = nc.dram_tensor("wk", [D, DC], dtr, kind="ExternalInput").ap()
    wv = nc.dram_tensor("wv", [D, DC], dtr, kind="ExternalInput").ap()
    wo = nc.dram_tensor("wo", [D, D], dtr, kind="ExternalInput").ap()
    cosd = nc.dram_tensor("cosd", [128, T], mybir.dt.float16, kind="ExternalInput").ap()
    sind = nc.dram_tensor("sind", [128, T], mybir.dt.float16, kind="ExternalInput").ap()
    mskd = nc.dram_tensor("mskd", [128, 4 * TS], mybir.dt.bfloat16,
                          kind="ExternalInput").ap()
    onesd = nc.dram_tensor("onesd", [128, 128], dtr, kind="ExternalInput").ap()
    pswapd = nc.dram_tensor("pswapd", [128, 128], dtr, kind="ExternalInput").ap()
    outp = nc.dram_tensor("out", [D, TS], dtf, kind="ExternalOutput").ap()

    with tile.TileContext(nc) as tc:
        with tc.tile_pool(name="const", bufs=1) as constp, \
             tc.tile_pool(name="big", bufs=1) as bigp, \
             tc.tile_pool(name="xt", bufs=2) as xtp, \
             tc.tile_pool(name="rt", bufs=1) as rtp, \
             tc.tile_pool(name="pt", bufs=2) as ptp, \
             tc.tile_pool(name="dv", bufs=1) as dvp, \
             tc.tile_pool(name="cc", bufs=16) as ccp, \
             tc.tile_pool(name="wop", bufs=4) as wop, \
             tc.tile_pool(name="ot", bufs=1) as otp, \
             tc.tile_pool(name="ps", bufs=4, space="PSUM") as psp, \
             tc.tile_pool(name="ps2", bufs=2, space="PSUM") as ps2p, \
             tc.tile_pool(name="dram", bufs=1, space="DRAM") as dramp:

            # ---- constants -> SBUF ----
            wq_s = constp.tile([128, KC * DC], dtr)
            wk_s = constp.tile([128, KC * DC], dtr)
            wv_s = constp.tile([128, KC * DC], dtr)
            for dst, src in ((wq_s, wq), (wk_s, wk), (wv_s, wv)):
                nc.sync.dma_start(
                    dst[:].rearrange("p (k m) -> p k m", k=KC),
                    src.rearrange("(k p) m -> p k m", p=128))
            cos_s = constp.tile([128, T], mybir.dt.float16)
            sin_s = constp.tile([128, T], mybir.dt.float16)
            nc.sync.dma_start(cos_s[:], cosd[:, :])
            nc.sync.dma_start(sin_s[:], sind[:, :])
            msk_s = constp.tile([128, 4 * TS], mybir.dt.bfloat16)
            nc.sync.dma_start(msk_s[:], mskd[:, :])
            ones_s = constp.tile([128, 128], dtr)
            nc.sync.dma_start(ones_s[:], onesd[:, :])
            psw_s = constp.tile([128, 128], dtr)
            nc.sync.dma_start(psw_s[:], pswapd[:, :])

            qd = bigp.tile([128, HPC * BT], dtr)
            kd = bigp.tile([128, HPC * BT], dtr)
            vs = bigp.tile([128, (BT // 128) * DC], dtr)

            send1 = dramp.tile([NCORES * 128, TS], dtr)
            recv1 = dramp.tile([NCORES * 128, TS], dtr)
            send2 = dramp.tile([NCORES * 128, TS], dtr)
            recv2 = dramp.tile([NCORES * 128, TS], dtr)

            # ---- phase B: projections + RoPE (256-wide t-supers so each
            # accumulator owns a full PSUM tile; interleaved accumulation
            # groups inside one bank corrupt each other) ----
            PTS = 256
            for ts in range(BT // PTS):
                pos0 = (ts % (T // PTS)) * PTS     # position within batch
                psq = [psp.tile([128, PTS], dtf, tag="mm", name=f"psq{_h}")
                       for _h in range(2)]
                psk = [psp.tile([128, PTS], dtf, tag="mm", name=f"psk{_h}")
                       for _h in range(2)]
                psv = [ps2p.tile([128, PTS], dtf, tag="acc", name=f"psv{_h}")
                       for _h in range(2)]
                for k in range(KC):
                    xt = xtp.tile([128, PTS], dtr)
                    nc.sync.dma_start(
                        xt[:], xT[k * 128:(k + 1) * 128, ts * PTS:(ts + 1) * PTS])
                    st, sp = (k == 0), (k == KC - 1)
                    for hl in range(2):
                        wq_c = wq_s[:, k * DC + hl * 128: k * DC + (hl + 1) * 128]
                        wk_c = wk_s[:, k * DC + hl * 128: k * DC + (hl + 1) * 128]
                        nc.tensor.matmul(psq[hl][:], wq_c, xt[:], start=st, stop=sp)
                        nc.tensor.matmul(psk[hl][:], wk_c, xt[:], start=st, stop=sp)
                    for tb in range(2):
                        nc.tensor.matmul(
                            psv[tb][:], xt[:, tb * 128:(tb + 1) * 128],
                            wv_s[:, k * DC:(k + 1) * DC], start=st, stop=sp)
                # V eviction: [128, 256] copies into vs (t-block tbg = 2*ts+tb)
                for tb in range(2):
                    tbg = ts * 2 + tb
                    nc.scalar.copy(vs[:, tbg * DC:(tbg + 1) * DC], psv[tb][:])
                # RoPE: o = psl*cos + swap(psl)*[-sin;sin]
                for psl, dst in ((psq, qd), (psk, kd)):
                    for hl in range(2):
                        tmp = rtp.tile([128, PTS], dtr)
                        nc.scalar.copy(tmp[:], psl[hl][:])
                        psr = ps2p.tile([128, PTS], dtf, tag="rot")
                        nc.tensor.matmul(psr[:], psw_s[:], tmp[:],
                                         start=True, stop=True)
                        odst = dst[:, hl * BT + ts * PTS: hl * BT + (ts + 1) * PTS]
                        nc.vector.tensor_mul(odst, psl[hl][:],
                                             cos_s[:, pos0:pos0 + PTS])
                        nc.vector.tensor_mul(psr[:], psr[:],
                                             sin_s[:, pos0:pos0 + PTS])
                        nc.vector.tensor_add(odst, odst, psr[:])

            # ---- phase C: attention, pair order (b0,h0),(b1,h0) | (b0,h1),(b1,h1)
            for hl in range(2):
                for b in range(B):
                    qh0 = hl * BT + b * T
                    for R in range(NRS):
                        ps_ctx = ps2p.tile([128, TS], dtf, tag="acc")
                        ps_den = ps2p.tile([128, TS], dtf, tag="rot")
                        ntb = 4 * (R + 1)
                        for tb in range(ntb):
                            ps_s = psp.tile([128, TS], dtf, tag="mm")
                            nc.tensor.matmul(
                                ps_s[:],
                                kd[:, qh0 + tb * 128: qh0 + (tb + 1) * 128],
                                qd[:, qh0 + R * TS: qh0 + (R + 1) * TS],
                                start=True, stop=True)
                            pt = ptp.tile([128, TS], dtr)
                            nc.scalar.activation(
                                pt[:], ps_s[:], mybir.ActivationFunctionType.Exp,
                                scale=SCALE)
                            j = tb - 4 * R
                            if j >= 0:
                                nc.vector.tensor_mul(
                                    pt[:], pt[:], msk_s[:, j * TS:(j + 1) * TS])
                            tbg = b * (T // 128) + tb
                            vh = vs[:, tbg * DC + hl * 128: tbg * DC + (hl + 1) * 128]
                            st, sp = (tb == 0), (tb == ntb - 1)
                            nc.tensor.matmul(ps_ctx[:], vh, pt[:], start=st, stop=sp)
                            nc.tensor.matmul(ps_den[:], ones_s[:], pt[:],
                                             start=st, stop=sp)
                        rc = dvp.tile([128, TS], dtf)
                        nc.vector.reciprocal(rc[:], ps_den[:])
                        cx = dvp.tile([128, TS], dtr)
                        nc.vector.tensor_mul(cx[:], ps_ctx[:], rc[:])
                        jblk = b * NRS + R
                        sendb = send1 if hl == 0 else send2
                        nc.sync.dma_start(
                            sendb[jblk * 128:(jblk + 1) * 128, :], cx[:])
                # A2A for this head-slot
                sendb, recvb = (send1, recv1) if hl == 0 else (send2, recv2)
                nc.gpsimd.collective_compute(
                    "AllToAll", mybir.AluOpType.bypass,
                    replica_groups=[list(range(NCORES))],
                    ins=[sendb.opt()], outs=[recvb.opt()])

            if dbg:
                for nm, src_t in (("dqd", qd), ("dkd", kd), ("dvs", vs)):
                    nc.sync.dma_start(dbg_t[nm].bitcast(dtr), src_t[:])
                for nm, src_t in (("dsend1", send1), ("dsend2", send2),
                                  ("drecv1", recv1), ("drecv2", recv2)):
                    nc.sync.dma_start(dbg_t[nm].bitcast(dtr), src_t[:])

            # ---- phase D: output projection (t-sharded, full Wo) ----
            ctx_t = []
            for g in range(KC):       # global cd chunk = head g
                src = recv1 if g % 2 == 0 else recv2
                c = g // 2
                t_ = ccp.tile([128, TS], dtr, tag="cc")
                nc.sync.dma_start(t_[:], src[c * 128:(c + 1) * 128, :])
                ctx_t.append(t_)
            for oc in range(KC):
                ps_o = psp.tile([128, TS], dtf, tag="mm")
                for g in range(KC):
                    wo_t = wop.tile([128, 128], dtr)
                    nc.sync.dma_start(
                        wo_t[:],
                        wo[g * 128:(g + 1) * 128, oc * 128:(oc + 1) * 128])
                    nc.tensor.matmul(
                        ps_o[:], wo_t[:], ctx_t[g][:],
                        start=(g == 0), stop=(g == KC - 1))
                ot = otp.tile([128, TS], dtf)
                nc.scalar.copy(ot[:], ps_o[:])
                nc.sync.dma_start(outp[oc * 128:(oc + 1) * 128, :], ot[:])

    nc.compile()
    _cache[key] = nc
    return nc


def host_prep(x, Wq, Wk, Wv, Wo):
    x = np.asarray(x, dtype=np.float32)
    Wq = np.asarray(Wq, dtype=np.float32)
    Wk = np.asarray(Wk, dtype=np.float32)
    Wv = np.asarray(Wv, dtype=np.float32)
    Wo = np.asarray(Wo, dtype=np.float32)

    xT = np.ascontiguousarray(x.reshape(BT, D).T)
    perm = np.concatenate([np.arange(0, DH, 2), np.arange(1, DH, 2)])

    pos = np.arange(T, dtype=np.float64)
    inv = ROPE_THETA ** (-np.arange(0, DH, 2, dtype=np.float64) / DH)  # [64]
    ang = inv[:, None] * pos[None, :]                                  # [64, T]
    c64 = np.cos(ang)
    s64 = np.sin(ang)
    cos128 = np.concatenate([c64, c64], axis=0).astype(np.float16)   # [128, T]
    sin128 = np.concatenate([-s64, s64], axis=0).astype(np.float16)  # [-sin; sin]

    tl = np.arange(128)[:, None]
    rl = np.arange(TS)[None, :]
    msk = np.concatenate(
        [(tl + 128 * j <= rl).astype(np.float32) for j in range(4)],
        axis=1).astype(ml_dtypes.bfloat16)                     # [128, 4*TS]

    ones = np.ones((128, 128), dtype=np.float32)
    pswap = np.zeros((128, 128), dtype=np.float32)
    pswap[(np.arange(128) + 64) % 128, np.arange(128)] = 1.0

    in_maps = []
    for i in range(NCORES):
        idx = np.concatenate([i * DC + h * DH + perm for h in range(HPC)])
        in_maps.append({
            "xT": xT,
            "wq": np.ascontiguousarray(Wq[:, idx]),
            "wk": np.ascontiguousarray(Wk[:, idx]),
            "wv": np.ascontiguousarray(Wv[:, i * DC:(i + 1) * DC]),
            "wo": Wo,
            "cosd": cos128, "sind": sin128, "mskd": msk,
            "onesd": ones, "pswapd": pswap,
        })
    return in_maps


def assemble(results):
    out_T = np.concatenate([results[i]["out"] for i in range(NCORES)], axis=1)
    return np.ascontiguousarray(out_T.T).reshape(B, T, D).astype(np.float32)


def kernel(x, Wq, Wk, Wv, Wo):
    nc = build()
    in_maps = host_prep(x, Wq, Wk, Wv, Wo)
    r = bass_utils.run_bass_kernel_spmd(nc, in_maps,
                                        core_ids=list(range(NCORES)))
    return assemble(r.results)
